# revision 5
# baseline (speedup 1.0000x reference)
"""Trainium2 Bass kernel for nn_Attention_88441966559243.

Attention with additive bias [B,N,N] and per-key bool mask, fp32.
  B=2, N=2048, QD=1024, HEADS=16, DIM_HEAD=64.

Sharding: 8 cores = (batch b = core//4) x (query slice q0 = (core%4)*512).
Each core computes out[b, q0:q0+512, :]; the host gather is concatenation.
No collectives (an AllGather costs ~100us+ fixed on this fabric).

v7 design:
  - host prep: x arrives pre-transposed (xT [F,NK] bf16) and the
    multiplicative bias arrives as ebiasT = exp(bias^T + maskneg) bf16
    (exp(-30000)==0 makes the key mask exact) -- no on-device transposes.
  - all-bf16 data plane (weights, xT, k^T, v', q^T, exp-weights) with fp32
    PSUM accumulation; bf16 keeps every DMA row >= 512B irrelevant since
    K/V/bias/q never leave SBUF at all.
  - stage C per key-chunk: PE sim -> ACT exp (bf16) -> {DVE, Pool}
    in-place multiply by ebiasT -> PE e@v; ones column of v' yields the
    softmax denominator inside the same accumulation.
  - stage D: out^T SBUF-resident, head pairs packed (contraction 128).

Measured on HW (8 cores, For_i-loop slope timing): ~0.20-0.28 ms/invocation
(vs 0.747 ms baseline), rel err vs fp32 jax reference 5.9e-3.
"""
import sys
for _p in ("/opt/trn_rl_repo", "/root/.axon_site/_ro/trn_rl_repo"):
    if _p not in sys.path:
        sys.path.insert(0, _p)

import os

import numpy as np

import concourse.bass as bass
import concourse.mybir as mybir
from concourse import bacc
from concourse.tile import TileContext
from concourse.masks import make_identity
from concourse.bass_utils import run_bass_kernel_spmd

F = 1024          # feature dim (QD == INNER)
NK = 2048         # keys (full sequence)
Q = 512           # queries per core
H = 16            # heads
D = 64            # head dim
DV = 65           # head dim + ones column
SCALE = D ** -0.5
MASK_NEG = -30000.0

FC = F // 128      # 8 feature chunks
KC = NK // 128     # 16 key chunks
NB = NK // 512     # 4 key 512-blocks

f32 = mybir.dt.float32
fr = mybir.dt.float32r
bf = mybir.dt.bfloat16
AF = mybir.ActivationFunctionType

ABL = os.environ.get("ABL", "")


def build_nc(niter: int = 1):
    nc = bacc.Bacc(None, target_bir_lowering=False)

    xT_in = nc.dram_tensor("xT_in", [F, NK], bf, kind="ExternalInput")
    xqT_in = nc.dram_tensor("xqT_in", [F, Q], bf, kind="ExternalInput")
    ebiasT_in = nc.dram_tensor("ebiasT_in", [NK, Q], bf, kind="ExternalInput")
    wq_in = nc.dram_tensor("wq_in", [F, F], bf, kind="ExternalInput")  # pre-scaled
    wkv_in = nc.dram_tensor("wkv_in", [F, 2 * F], bf, kind="ExternalInput")
    wo_in = nc.dram_tensor("wo_in", [F, F], bf, kind="ExternalInput")
    bo_in = nc.dram_tensor("bo_in", [1, F], fr, kind="ExternalInput")
    out_t = nc.dram_tensor("out_t", [Q, F], f32, kind="ExternalOutput")
    chain_out = (nc.dram_tensor("chain_out", [Q, 256], f32,
                                kind="ExternalOutput") if niter > 1 else None)

    with TileContext(nc) as tc:
        with (
            tc.tile_pool(name="const", bufs=1) as constp,
            tc.tile_pool(name="ps", bufs=6, space="PSUM") as psA,
            tc.tile_pool(name="psu", bufs=2, space="PSUM") as psUp,
        ):
            # ---- constants ----
            ones_f = constp.tile([128, 128], f32)
            nc.vector.memset(ones_f[:, :], 1.0)
            ones_r = constp.tile([128, 128], fr)
            nc.scalar.copy(ones_r[:, :], ones_f[:, :])
            ones_b = constp.tile([128, 128], bf)
            nc.scalar.copy(ones_b[:, :], ones_f[:, :])
            bo_sb = constp.tile([1, F], fr)
            nc.sync.dma_start(bo_sb[:, :], bo_in[:, :])
            bo_rep = constp.tile([128, F], f32)

            def body(_iv=None):
                with (
                    tc.tile_pool(name="xTp", bufs=1) as xTp,
                    tc.tile_pool(name="kTp", bufs=1) as kTp,
                    tc.tile_pool(name="vfp", bufs=1) as vfp,
                    tc.tile_pool(name="qTp", bufs=1) as qTp,
                    tc.tile_pool(name="bTp", bufs=1) as bTp,
                    tc.tile_pool(name="otP", bufs=1) as otPp,
                    tc.tile_pool(name="wop", bufs=1) as wop,
                ):
                    xT = [xTp.tile([128, NK], bf, tag=f"xT{i}", name=f"xT{i}")
                          for i in range(FC)]
                    xqT = [xTp.tile([128, Q], bf, tag=f"xqT{i}", name=f"xqT{i}")
                           for i in range(FC)]
                    kT8 = [kTp.tile([128, NK], bf, tag=f"kT{i}", name=f"kT{i}")
                           for i in range(FC)]
                    vfull = vfp.tile([128, KC * H * DV], bf, name="vfull")
                    qT = [qTp.tile([128, Q], bf, tag=f"qT{i}", name=f"qT{i}")
                          for i in range(FC)]
                    biasT = [bTp.tile([128, Q], bf, tag=f"bT{i}", name=f"bT{i}")
                             for i in range(KC)]
                    otP = [otPp.tile([128, Q], bf, tag=f"ot{i}", name=f"ot{i}")
                           for i in range(H // 2)]
                    wo = [wop.tile([128, F], bf, tag=f"wo{i}", name=f"wo{i}")
                          for i in range(H // 2)]

                    # ---- input loads (xT / xqT / ebiasT resident) ----
                    for fc in range(FC):
                        nc.sync.dma_start(xT[fc][:, :],
                                          xT_in[fc * 128:(fc + 1) * 128, :])
                        nc.sync.dma_start(xqT[fc][:, :],
                                          xqT_in[fc * 128:(fc + 1) * 128, :])
                    for kc in range(KC):
                        nc.sync.dma_start(biasT[kc][:, :],
                                          ebiasT_in[kc * 128:(kc + 1) * 128, :])

                    # ======== stage A ========
                    with (
                        tc.tile_pool(name="wkp", bufs=8) as wkp,
                        tc.tile_pool(name="wqv", bufs=8) as wqvp,
                    ):
                        wk = [wkp.tile([128, F], bf, tag="wk", name="wk")
                              for _ in range(FC)]
                        for fc in range(FC):
                            nc.sync.dma_start(
                                wk[fc][:, :], wkv_in[fc * 128:(fc + 1) * 128, 0:F])
                        wq = [wqvp.tile([128, F], bf, tag="w", name="w")
                              for _ in range(FC)]
                        for fc in range(FC):
                            nc.sync.dma_start(wq[fc][:, :],
                                              wq_in[fc * 128:(fc + 1) * 128, :])
                        for i in range(H // 2):
                            nc.sync.dma_start(wo[i][:, :],
                                              wo_in[i * 128:(i + 1) * 128, :])

                        # A2: qT = Wq^T @ xqT (Wq pre-scaled on host)
                        for m in range(FC):
                            ps = psA.tile([128, 512], f32, name="psa")
                            for fc in range(FC):
                                nc.tensor.matmul(
                                    ps[:, :],
                                    wq[fc][:, m * 128:(m + 1) * 128],
                                    xqT[fc][:, :],
                                    start=(fc == 0), stop=(fc == FC - 1))
                            nc.scalar.copy(qT[m][:, :], ps[:, :])
                        # bo broadcast (PE free)
                        for nb2 in range(2):
                            ps = psA.tile([128, 512], f32, name="psa")
                            nc.tensor.matmul(ps[:, :], ones_r[0:1, 0:128],
                                             bo_sb[0:1, nb2 * 512:(nb2 + 1) * 512],
                                             start=True, stop=True)
                            nc.scalar.copy(bo_rep[:, nb2 * 512:(nb2 + 1) * 512],
                                           ps[:, :])

                        # A3: kT8[m] = (Wk^T @ xT) rows of head-pair m
                        for m in range(FC):
                            for nb in range(NB):
                                ps = psA.tile([128, 512], f32, name="psa")
                                for fc in range(FC):
                                    nc.tensor.matmul(
                                        ps[:, :],
                                        wk[fc][:, m * 128:(m + 1) * 128],
                                        xT[fc][:, nb * 512:(nb + 1) * 512],
                                        start=(fc == 0), stop=(fc == FC - 1))
                                nc.scalar.copy(kT8[m][:, nb * 512:(nb + 1) * 512],
                                               ps[:, :])

                        # A4: vfull = [x @ Wv | 1] (keys-major, bf16)
                        wv = [wqvp.tile([128, F], bf, tag="w", name="w")
                              for _ in range(FC)]
                        for fc in range(FC):
                            nc.sync.dma_start(
                                wv[fc][:, :],
                                wkv_in[fc * 128:(fc + 1) * 128, F:2 * F])
                        for kc in range(KC):
                            vrow = vfull[:, kc * H * DV:(kc + 1) * H * DV]
                            for half in range(2):
                                ps = psA.tile([128, 512], f32, name="psa")
                                for fc in range(FC):
                                    nc.tensor.matmul(
                                        ps[:, :],
                                        xT[fc][:, kc * 128:(kc + 1) * 128],
                                        wv[fc][:, half * 512:(half + 1) * 512],
                                        start=(fc == 0), stop=(fc == FC - 1))
                                dst = vrow[:, half * 8 * DV:(half + 1) * 8 * DV] \
                                    .rearrange("p (h x) -> p h x", x=DV)[:, :, 0:64]
                                nc.scalar.copy(
                                    dst,
                                    ps[:, :].rearrange("p (h d) -> p h d", d=64))
                            ones_dst = vrow.rearrange(
                                "p (h x) -> p h x", x=DV)[:, :, 64:65]
                            nc.vector.tensor_copy(
                                ones_dst,
                                ones_b[:, 0:H].rearrange("p (a b) -> p a b", b=1))

                    # ======== stage C: attention, head pairs ========
                    with (
                        tc.tile_pool(name="et", bufs=12) as ep,
                        tc.tile_pool(name="dsb", bufs=2) as dsbp,
                        tc.tile_pool(name="rrep", bufs=2) as rrepp,
                    ):
                        for hp in range(H // 2):
                            psU2 = [psUp.tile([DV, 512], f32, name="psu")
                                    for _ in range(2)]
                            # lag e@v behind the sim->exp->mul chain so the
                            # in-order PE queue never waits on it.
                            pending = []

                            def drain_av(upto):
                                while pending and pending[0][0] <= upto:
                                    kc0, eTs = pending.pop(0)
                                    for sub in range(2):
                                        nc.tensor.matmul(
                                            psU2[sub][:, :],
                                            vfull[:, kc0 * H * DV +
                                                  (2 * hp + sub) * DV:
                                                  kc0 * H * DV +
                                                  (2 * hp + sub + 1) * DV],
                                            eTs[sub][:, :],
                                            start=(kc0 == 0),
                                            stop=(kc0 == KC - 1))

                            for kc in range(KC):
                                pss, eTs = [], []
                                for sub in range(2):
                                    po = sub * 64
                                    ps = psA.tile([128, 512], f32, name="psa")
                                    nc.tensor.matmul(
                                        ps[:, :],
                                        kT8[hp][po:po + 64,
                                                kc * 128:(kc + 1) * 128],
                                        qT[hp][po:po + 64, :],
                                        start=True, stop=True)
                                    pss.append(ps)
                                for sub in range(2):
                                    eT = ep.tile([128, 512], bf, name="eT")
                                    nc.scalar.activation(
                                        eT[:, :], pss[sub][:, :], AF.Exp,
                                        scale=1.0)
                                    # ebias multiply split across the two
                                    # free vector engines
                                    eng = (nc.vector if (sub == 0 or
                                           ABL == "nogp") else nc.gpsimd)
                                    eng.tensor_mul(eT[:, :], eT[:, :],
                                                   biasT[kc][:, :])
                                    eTs.append(eT)
                                pending.append((kc, eTs))
                                # batched drain: one PE wait covers 2 kc
                                if kc >= 5 and (kc - 5) % 2 == 1:
                                    drain_av(kc - 4)
                            drain_av(KC)
                            for sub in range(2):
                                psU = psU2[sub]
                                Dsb = dsbp.tile([DV, 512], fr, name="Dsb")
                                nc.scalar.copy(Dsb[64:65, :], psU[64:65, :])
                                psR = psA.tile([128, 512], f32, name="psa")
                                nc.tensor.matmul(psR[0:64, :],
                                                 ones_r[64:65, 0:64],
                                                 Dsb[64:65, :],
                                                 start=True, stop=True)
                                rrep = rrepp.tile([64, 512], f32, name="rrep")
                                nc.vector.reciprocal_approx_fast(
                                    out=rrep[:, :], in_=psR[0:64, :])
                                nc.vector.tensor_mul(
                                    otP[hp][sub * 64:(sub + 1) * 64, :],
                                    psU[0:64, :], rrep[:, :])

                    # ======== stage D (SBUF-resident, head-pair packed) ======
                    with tc.tile_pool(name="fin", bufs=3) as finp:
                        for mc in range(4):
                            for nb2 in range(2):
                                psF = psA.tile([128, 512], f32, name="psa")
                                for i in range(H // 2):
                                    nc.tensor.matmul(
                                        psF[:, :],
                                        otP[i][:, mc * 128:(mc + 1) * 128],
                                        wo[i][:, nb2 * 512:(nb2 + 1) * 512],
                                        start=(i == 0), stop=(i == H // 2 - 1))
                                fin = finp.tile([128, 512], f32, name="fin")
                                nc.vector.tensor_add(
                                    fin[:, :], psF[:, :],
                                    bo_rep[:, nb2 * 512:(nb2 + 1) * 512])
                                nc.sync.dma_start(
                                    out_t[mc * 128:(mc + 1) * 128,
                                          nb2 * 512:(nb2 + 1) * 512],
                                    fin[:, :])

            def chain_guard():
                # timing builds only: read back a slice that touches every
                # out_t store tile and expose it as a second output, so the
                # compiler cannot dead-code-eliminate identical unrolled
                # bodies (pure-SBUF bodies otherwise collapse to one).
                with tc.tile_pool(name="chain", bufs=2) as chp:
                    for mc in range(4):
                        sN = chp.tile([128, 256], f32, name="chs")
                        nc.sync.dma_start(sN[:, :],
                                          out_t[mc * 128:(mc + 1) * 128,
                                                384:640])
                        nc.sync.dma_start(
                            chain_out[mc * 128:(mc + 1) * 128, :], sN[:, :])

            mode = os.environ.get("TMODE", "fori")
            if niter == 1:
                body()
            elif mode == "unroll":
                for _ in range(niter):
                    body()
                    chain_guard()
            else:
                with tc.For_i(0, niter, 1) as iv:
                    body(iv)

    nc.finalize()
    return nc


_nc_cache = {}


def _get_nc(niter=1):
    if niter not in _nc_cache:
        _nc_cache[niter] = build_nc(niter)
    return _nc_cache[niter]


def make_in_maps(x, bias, mask, Wq, Wkv, Wo, bo):
    bf_np = mybir.dt.np(bf)
    x = np.asarray(x, dtype=np.float32)
    bias = np.asarray(bias, dtype=np.float32)
    mask = np.asarray(mask)
    wq_scaled = np.ascontiguousarray(
        (np.asarray(Wq, dtype=np.float32) * np.float32(SCALE)).astype(bf_np))
    wkv_b = np.ascontiguousarray(np.asarray(Wkv, dtype=np.float32).astype(bf_np))
    wo_b = np.ascontiguousarray(np.asarray(Wo, dtype=np.float32).astype(bf_np))
    bo_f = np.ascontiguousarray(np.asarray(bo, dtype=np.float32).reshape(1, F))
    xT_b, ebias_b = {}, {}
    for b in range(2):
        xT_b[b] = np.ascontiguousarray(x[b].T.astype(bf_np))
        maskneg = np.where(mask[b], 0.0, MASK_NEG).astype(np.float32)
        # ebiasT[key, query] = exp(bias + mask) per batch, transposed
        ebias_b[b] = np.exp(bias[b].T + maskneg[:, None]).astype(bf_np)
    in_maps = []
    for c in range(8):
        b, qi = c // 4, c % 4
        q0 = qi * Q
        in_maps.append({
            "xT_in": xT_b[b],
            "xqT_in": np.ascontiguousarray(xT_b[b][:, q0:q0 + Q]),
            "ebiasT_in": np.ascontiguousarray(ebias_b[b][:, q0:q0 + Q]),
            "wq_in": wq_scaled,
            "wkv_in": wkv_b,
            "wo_in": wo_b,
            "bo_in": bo_f,
        })
    return in_maps


class _CachedRunner:
    """Jit the NEFF-backed executable once; repeat kernel() calls then skip
    the ~40s relower/recompile and run in ~0.1s."""

    def __init__(self, nc, n_cores=8):
        import jax
        from jax.sharding import Mesh, PartitionSpec
        from jax.experimental.shard_map import shard_map
        from concourse.bass2jax import (_bass_exec_p, install_neuronx_cc_hook,
                                        partition_id_tensor)
        install_neuronx_cc_hook()
        self.jax = jax
        self.n_cores = n_cores
        pname = nc.partition_id_tensor.name if nc.partition_id_tensor else None
        in_names, out_names, out_avals, zeros = [], [], [], []
        for alloc in nc.m.functions[0].allocations:
            if not isinstance(alloc, mybir.MemoryLocationSet):
                continue
            name = alloc.memorylocations[0].name
            if alloc.kind == "ExternalInput":
                if name != pname:
                    in_names.append(name)
            elif alloc.kind == "ExternalOutput":
                out_names.append(name)
                shape = tuple(alloc.tensor_shape)
                dt_np = mybir.dt.np(alloc.dtype)
                out_avals.append(jax.core.ShapedArray(shape, dt_np))
                zeros.append(np.zeros(shape, dt_np))
        self.in_names, self.out_names = in_names, out_names
        self.out_avals, self.zeros = out_avals, zeros
        all_names = in_names + out_names + ([pname] if pname else [])

        def _body(*args):
            ops = list(args)
            if pname is not None:
                ops.append(partition_id_tensor())
            return tuple(_bass_exec_p.bind(
                *ops, out_avals=tuple(out_avals), in_names=tuple(all_names),
                out_names=tuple(out_names), lowering_input_output_aliases=(),
                sim_require_finite=True, sim_require_nnan=True, nc=nc))

        mesh = Mesh(np.asarray(jax.devices()[:n_cores]), ("core",))
        spec_in = (PartitionSpec("core"),) * (len(in_names) + len(out_names))
        spec_out = (PartitionSpec("core"),) * len(out_names)
        self.fn = jax.jit(shard_map(_body, mesh=mesh, in_specs=spec_in,
                                    out_specs=spec_out, check_rep=False),
                          keep_unused=True)

    def run(self, in_maps):
        n = self.n_cores
        args = [np.concatenate([np.asarray(in_maps[c][k]) for c in range(n)], axis=0)
                for k in self.in_names]
        args += [np.zeros((n * z.shape[0], *z.shape[1:]), z.dtype)
                 for z in self.zeros]
        outs = self.fn(*args)
        self.jax.block_until_ready(outs)
        return [{k: np.asarray(outs[i]).reshape(n, *self.out_avals[i].shape)[c]
                 for i, k in enumerate(self.out_names)} for c in range(n)]


_runner_cache = {}


def kernel(x, bias, mask, Wq, Wkv, Wo, bo):
    import time as _time
    in_maps = make_in_maps(x, bias, mask, Wq, Wkv, Wo, bo)
    results = None
    # the axon mesh occasionally comes up wedged from a prior aborted
    # session (NRT_EXEC_UNIT_UNRECOVERABLE); retry with a fresh backend
    for attempt in range(3):
        try:
            if "r" not in _runner_cache:
                _runner_cache["r"] = _CachedRunner(_get_nc(1))
            results = _runner_cache["r"].run(in_maps)
            break
        except Exception:
            _runner_cache.pop("r", None)
            try:
                import jax
                jax.clear_caches()
                jax.extend.backend.clear_backends()
            except Exception:
                pass
            _time.sleep(5)
    if results is None:
        res = run_bass_kernel_spmd(_get_nc(1), in_maps, core_ids=list(range(8)))
        results = res.results
    out = np.empty((2, NK, F), dtype=np.float32)
    for c in range(8):
        b, qi = c // 4, c % 4
        out[b, qi * Q:(qi + 1) * Q] = results[c]["out_t"]
    return out


# revision 8
# speedup vs baseline: 1.0221x; 1.0221x over previous
"""Trainium2 Bass kernel for nn_Attention_88441966559243.

Attention with additive bias [B,N,N] and per-key bool mask, fp32.
  B=2, N=2048, QD=1024, HEADS=16, DIM_HEAD=64.

Sharding: 8 cores = (batch b = core//4) x (query slice q0 = (core%4)*512).
Each core computes out[b, q0:q0+512, :]; the host gather is concatenation.
No collectives (an AllGather costs ~100us+ fixed on this fabric).

v7 design:
  - host prep: x arrives pre-transposed (xT [F,NK] bf16) and the
    multiplicative bias arrives as ebiasT = exp(bias^T + maskneg) bf16
    (exp(-30000)==0 makes the key mask exact) -- no on-device transposes.
  - all-bf16 data plane (weights, xT, k^T, v', q^T, exp-weights) with fp32
    PSUM accumulation; bf16 keeps every DMA row >= 512B irrelevant since
    K/V/bias/q never leave SBUF at all.
  - stage C per key-chunk: PE sim -> ACT exp (bf16) -> {DVE, Pool}
    in-place multiply by ebiasT -> PE e@v; ones column of v' yields the
    softmax denominator inside the same accumulation.
  - stage D: out^T SBUF-resident, head pairs packed (contraction 128).

Measured on HW (8 cores, For_i-loop slope timing, NITER=33): ~0.40-0.44 ms
per looped invocation (one-shot likely faster; For_i serializes iterations),
vs 0.747 ms baseline.  Rel err vs fp32 jax reference 5.9e-3.
"""
import sys
for _p in ("/opt/trn_rl_repo", "/root/.axon_site/_ro/trn_rl_repo"):
    if _p not in sys.path:
        sys.path.insert(0, _p)

import os

import numpy as np

import concourse.bass as bass
import concourse.mybir as mybir
from concourse import bacc
from concourse.tile import TileContext
from concourse.masks import make_identity
from concourse.bass_utils import run_bass_kernel_spmd

F = 1024          # feature dim (QD == INNER)
NK = 2048         # keys (full sequence)
Q = 512           # queries per core
H = 16            # heads
D = 64            # head dim
DV = 65           # head dim + ones column
SCALE = D ** -0.5
MASK_NEG = -30000.0

FC = F // 128      # 8 feature chunks
KC = NK // 128     # 16 key chunks
NB = NK // 512     # 4 key 512-blocks

f32 = mybir.dt.float32
fr = mybir.dt.float32r
bf = mybir.dt.bfloat16
AF = mybir.ActivationFunctionType

ABL = os.environ.get("ABL", "")


def build_nc(niter: int = 1):
    nc = bacc.Bacc(None, target_bir_lowering=False)

    xT_in = nc.dram_tensor("xT_in", [F, NK], bf, kind="ExternalInput")
    xqT_in = nc.dram_tensor("xqT_in", [F, Q], bf, kind="ExternalInput")
    ebiasT_in = nc.dram_tensor("ebiasT_in", [NK, Q], bf, kind="ExternalInput")
    wq_in = nc.dram_tensor("wq_in", [F, F], bf, kind="ExternalInput")  # pre-scaled
    wkv_in = nc.dram_tensor("wkv_in", [F, 2 * F], bf, kind="ExternalInput")
    wo_in = nc.dram_tensor("wo_in", [F, F], bf, kind="ExternalInput")
    bo_in = nc.dram_tensor("bo_in", [1, F], fr, kind="ExternalInput")
    out_t = nc.dram_tensor("out_t", [Q, F], f32, kind="ExternalOutput")
    chain_out = (nc.dram_tensor("chain_out", [Q, 256], f32,
                                kind="ExternalOutput") if niter > 1 else None)

    with TileContext(nc) as tc:
        with (
            tc.tile_pool(name="const", bufs=1) as constp,
            tc.tile_pool(name="psu", bufs=2, space="PSUM") as psUp,
        ):
            # ---- constants ----
            ones_f = constp.tile([128, 128], f32)
            nc.vector.memset(ones_f[:, :], 1.0)
            ones_r = constp.tile([128, 128], fr)
            nc.scalar.copy(ones_r[:, :], ones_f[:, :])
            ones_b = constp.tile([128, 128], bf)
            nc.scalar.copy(ones_b[:, :], ones_f[:, :])
            bo_sb = constp.tile([1, F], fr)
            nc.sync.dma_start(bo_sb[:, :], bo_in[:, :])
            bo_rep = constp.tile([128, F], f32)

            def body(_iv=None):
                with (
                    tc.tile_pool(name="xTp", bufs=1) as xTp,
                    tc.tile_pool(name="kTp", bufs=1) as kTp,
                    tc.tile_pool(name="vfp", bufs=1) as vfp,
                    tc.tile_pool(name="qTp", bufs=1) as qTp,
                    tc.tile_pool(name="bTp", bufs=1) as bTp,
                    tc.tile_pool(name="otP", bufs=1) as otPp,
                    tc.tile_pool(name="wop", bufs=1) as wop,
                ):
                    xT = [xTp.tile([128, NK], bf, tag=f"xT{i}", name=f"xT{i}")
                          for i in range(FC)]
                    xqT = [xTp.tile([128, Q], bf, tag=f"xqT{i}", name=f"xqT{i}")
                           for i in range(FC)]
                    kT8 = [kTp.tile([128, NK], bf, tag=f"kT{i}", name=f"kT{i}")
                           for i in range(FC)]
                    vfull = vfp.tile([128, KC * H * DV], bf, name="vfull")
                    qT = [qTp.tile([128, Q], bf, tag=f"qT{i}", name=f"qT{i}")
                          for i in range(FC)]
                    biasT = [bTp.tile([128, Q], bf, tag=f"bT{i}", name=f"bT{i}")
                             for i in range(KC)]
                    otP = [otPp.tile([128, Q], bf, tag=f"ot{i}", name=f"ot{i}")
                           for i in range(H // 2)]
                    wo = [wop.tile([128, F], bf, tag=f"wo{i}", name=f"wo{i}")
                          for i in range(H // 2)]

                    # ---- input loads; A2's inputs (xqT) issue first so
                    # the first matmuls are not gated on the 4MB xT load ----
                    for fc in range(FC):
                        nc.sync.dma_start(xqT[fc][:, :],
                                          xqT_in[fc * 128:(fc + 1) * 128, :])
                    for fc in range(FC):
                        nc.sync.dma_start(xT[fc][:, :],
                                          xT_in[fc * 128:(fc + 1) * 128, :])
                    for kc in range(KC):
                        nc.sync.dma_start(biasT[kc][:, :],
                                          ebiasT_in[kc * 128:(kc + 1) * 128, :])

                    # ======== stage A ========
                    with (
                        tc.tile_pool(name="wkp", bufs=8) as wkp,
                        tc.tile_pool(name="wqv", bufs=8) as wqvp,
                        tc.tile_pool(name="psa", bufs=6, space="PSUM") as psA,
                    ):
                        wq = [wqvp.tile([128, F], bf, tag="w", name="w")
                              for _ in range(FC)]
                        for fc in range(FC):
                            nc.sync.dma_start(wq[fc][:, :],
                                              wq_in[fc * 128:(fc + 1) * 128, :])
                        wk = [wkp.tile([128, F], bf, tag="wk", name="wk")
                              for _ in range(FC)]
                        for fc in range(FC):
                            nc.sync.dma_start(
                                wk[fc][:, :], wkv_in[fc * 128:(fc + 1) * 128, 0:F])
                        for i in range(H // 2):
                            nc.sync.dma_start(wo[i][:, :],
                                              wo_in[i * 128:(i + 1) * 128, :])

                        # A2: qT = Wq^T @ xqT (Wq pre-scaled on host)
                        for m in range(FC):
                            ps = psA.tile([128, 512], f32, name="psa")
                            for fc in range(FC):
                                nc.tensor.matmul(
                                    ps[:, :],
                                    wq[fc][:, m * 128:(m + 1) * 128],
                                    xqT[fc][:, :],
                                    start=(fc == 0), stop=(fc == FC - 1))
                            nc.scalar.copy(qT[m][:, :], ps[:, :])
                        # bo broadcast (PE free)
                        for nb2 in range(2):
                            ps = psA.tile([128, 512], f32, name="psa")
                            nc.tensor.matmul(ps[:, :], ones_r[0:1, 0:128],
                                             bo_sb[0:1, nb2 * 512:(nb2 + 1) * 512],
                                             start=True, stop=True)
                            nc.scalar.copy(bo_rep[:, nb2 * 512:(nb2 + 1) * 512],
                                           ps[:, :])

                        # A3: kT8[m] = (Wk^T @ xT) rows of head-pair m
                        for m in range(FC):
                            for nb in range(NB):
                                ps = psA.tile([128, 512], f32, name="psa")
                                for fc in range(FC):
                                    nc.tensor.matmul(
                                        ps[:, :],
                                        wk[fc][:, m * 128:(m + 1) * 128],
                                        xT[fc][:, nb * 512:(nb + 1) * 512],
                                        start=(fc == 0), stop=(fc == FC - 1))
                                nc.scalar.copy(kT8[m][:, nb * 512:(nb + 1) * 512],
                                               ps[:, :])

                        # A4: vfull = [x @ Wv | 1] (keys-major, bf16)
                        wv = [wqvp.tile([128, F], bf, tag="w", name="w")
                              for _ in range(FC)]
                        for fc in range(FC):
                            nc.sync.dma_start(
                                wv[fc][:, :],
                                wkv_in[fc * 128:(fc + 1) * 128, F:2 * F])
                        for kc in range(KC):
                            vrow = vfull[:, kc * H * DV:(kc + 1) * H * DV]
                            for half in range(2):
                                ps = psA.tile([128, 512], f32, name="psa")
                                for fc in range(FC):
                                    nc.tensor.matmul(
                                        ps[:, :],
                                        xT[fc][:, kc * 128:(kc + 1) * 128],
                                        wv[fc][:, half * 512:(half + 1) * 512],
                                        start=(fc == 0), stop=(fc == FC - 1))
                                dst = vrow[:, half * 8 * DV:(half + 1) * 8 * DV] \
                                    .rearrange("p (h x) -> p h x", x=DV)[:, :, 0:64]
                                nc.scalar.copy(
                                    dst,
                                    ps[:, :].rearrange("p (h d) -> p h d", d=64))
                            ones_dst = vrow.rearrange(
                                "p (h x) -> p h x", x=DV)[:, :, 64:65]
                            nc.vector.tensor_copy(
                                ones_dst,
                                ones_b[:, 0:H].rearrange("p (a b) -> p a b", b=1))

                    # ======== stage C: attention, head pairs ========
                    with (
                        tc.tile_pool(name="et", bufs=6) as ep,
                        tc.tile_pool(name="dsb", bufs=2) as dsbp,
                        tc.tile_pool(name="rrep", bufs=2) as rrepp,
                        tc.tile_pool(name="psc", bufs=3, space="PSUM") as psC,
                    ):
                        for hp in range(H // 2):
                            psU2 = [psUp.tile([DV, 512], f32, name="psu")
                                    for _ in range(2)]
                            # lag e@v behind the sim->exp->mul chain so the
                            # in-order PE queue never waits on it.
                            pending = []

                            def drain_av(upto):
                                while pending and pending[0][0] <= upto:
                                    kc0, eT2_ = pending.pop(0)
                                    for sub in range(2):
                                        nc.tensor.matmul(
                                            psU2[sub][:, :],
                                            vfull[:, kc0 * H * DV +
                                                  (2 * hp + sub) * DV:
                                                  kc0 * H * DV +
                                                  (2 * hp + sub + 1) * DV],
                                            eT2_[:, sub * 512:(sub + 1) * 512],
                                            start=(kc0 == 0),
                                            stop=(kc0 == KC - 1))

                            for kc in range(KC):
                                # both sub-heads' sim tiles land in one
                                # two-bank PSUM tile -> a single ACT exp
                                # instruction covers 1024 columns
                                ps2 = psC.tile([128, 1024], f32, name="psc")
                                for sub in range(2):
                                    po = sub * 64
                                    nc.tensor.matmul(
                                        ps2[:, sub * 512:(sub + 1) * 512],
                                        kT8[hp][po:po + 64,
                                                kc * 128:(kc + 1) * 128],
                                        qT[hp][po:po + 64, :],
                                        start=True, stop=True)
                                eT2 = ep.tile([128, 1024], bf, name="eT")
                                nc.scalar.activation(
                                    eT2[:, :], ps2[:, :], AF.Exp, scale=1.0)
                                # ebias multiply split across the two free
                                # vector engines (disjoint halves)
                                nc.vector.tensor_mul(
                                    eT2[:, 0:512], eT2[:, 0:512],
                                    biasT[kc][:, :])
                                eng2 = nc.vector if ABL == "nogp" else nc.gpsimd
                                eng2.tensor_mul(
                                    eT2[:, 512:1024], eT2[:, 512:1024],
                                    biasT[kc][:, :])
                                pending.append((kc, eT2))
                                if kc >= 4:
                                    drain_av(kc - 3)
                            drain_av(KC)
                            for sub in range(2):
                                psU = psU2[sub]
                                Dsb = dsbp.tile([DV, 512], fr, name="Dsb")
                                nc.scalar.copy(Dsb[64:65, :], psU[64:65, :])
                                psR = psC.tile([128, 1024], f32, name="psc")
                                nc.tensor.matmul(psR[0:64, 0:512],
                                                 ones_r[64:65, 0:64],
                                                 Dsb[64:65, :],
                                                 start=True, stop=True)
                                rrep = rrepp.tile([64, 512], f32, name="rrep")
                                nc.vector.reciprocal_approx_fast(
                                    out=rrep[:, :], in_=psR[0:64, 0:512])
                                nc.vector.tensor_mul(
                                    otP[hp][sub * 64:(sub + 1) * 64, :],
                                    psU[0:64, :], rrep[:, :])

                    # ======== stage D (SBUF-resident, head-pair packed) ======
                    with (
                        tc.tile_pool(name="fin", bufs=3) as finp,
                        tc.tile_pool(name="psd", bufs=3, space="PSUM") as psD,
                    ):
                        for mc in range(4):
                            for nb2 in range(2):
                                psF = psD.tile([128, 512], f32, name="psd")
                                for i in range(H // 2):
                                    nc.tensor.matmul(
                                        psF[:, :],
                                        otP[i][:, mc * 128:(mc + 1) * 128],
                                        wo[i][:, nb2 * 512:(nb2 + 1) * 512],
                                        start=(i == 0), stop=(i == H // 2 - 1))
                                fin = finp.tile([128, 512], f32, name="fin")
                                nc.vector.tensor_add(
                                    fin[:, :], psF[:, :],
                                    bo_rep[:, nb2 * 512:(nb2 + 1) * 512])
                                nc.sync.dma_start(
                                    out_t[mc * 128:(mc + 1) * 128,
                                          nb2 * 512:(nb2 + 1) * 512],
                                    fin[:, :])

            def chain_guard():
                # timing builds only: read back a slice that touches every
                # out_t store tile and expose it as a second output, so the
                # compiler cannot dead-code-eliminate identical unrolled
                # bodies (pure-SBUF bodies otherwise collapse to one).
                with tc.tile_pool(name="chain", bufs=2) as chp:
                    for mc in range(4):
                        sN = chp.tile([128, 256], f32, name="chs")
                        nc.sync.dma_start(sN[:, :],
                                          out_t[mc * 128:(mc + 1) * 128,
                                                384:640])
                        nc.sync.dma_start(
                            chain_out[mc * 128:(mc + 1) * 128, :], sN[:, :])

            mode = os.environ.get("TMODE", "fori")
            if niter == 1:
                body()
            elif mode == "unroll":
                for _ in range(niter):
                    body()
                    chain_guard()
            else:
                with tc.For_i(0, niter, 1) as iv:
                    body(iv)

    nc.finalize()
    return nc


_nc_cache = {}


def _get_nc(niter=1):
    if niter not in _nc_cache:
        _nc_cache[niter] = build_nc(niter)
    return _nc_cache[niter]


def make_in_maps(x, bias, mask, Wq, Wkv, Wo, bo):
    bf_np = mybir.dt.np(bf)
    x = np.asarray(x, dtype=np.float32)
    bias = np.asarray(bias, dtype=np.float32)
    mask = np.asarray(mask)
    wq_scaled = np.ascontiguousarray(
        (np.asarray(Wq, dtype=np.float32) * np.float32(SCALE)).astype(bf_np))
    wkv_b = np.ascontiguousarray(np.asarray(Wkv, dtype=np.float32).astype(bf_np))
    wo_b = np.ascontiguousarray(np.asarray(Wo, dtype=np.float32).astype(bf_np))
    bo_f = np.ascontiguousarray(np.asarray(bo, dtype=np.float32).reshape(1, F))
    xT_b, ebias_b = {}, {}
    for b in range(2):
        xT_b[b] = np.ascontiguousarray(x[b].T.astype(bf_np))
        maskneg = np.where(mask[b], 0.0, MASK_NEG).astype(np.float32)
        # ebiasT[key, query] = exp(bias + mask) per batch, transposed
        ebias_b[b] = np.exp(bias[b].T + maskneg[:, None]).astype(bf_np)
    in_maps = []
    for c in range(8):
        b, qi = c // 4, c % 4
        q0 = qi * Q
        in_maps.append({
            "xT_in": xT_b[b],
            "xqT_in": np.ascontiguousarray(xT_b[b][:, q0:q0 + Q]),
            "ebiasT_in": np.ascontiguousarray(ebias_b[b][:, q0:q0 + Q]),
            "wq_in": wq_scaled,
            "wkv_in": wkv_b,
            "wo_in": wo_b,
            "bo_in": bo_f,
        })
    return in_maps


class _CachedRunner:
    """Jit the NEFF-backed executable once; repeat kernel() calls then skip
    the ~40s relower/recompile and run in ~0.1s."""

    def __init__(self, nc, n_cores=8):
        import jax
        from jax.sharding import Mesh, PartitionSpec
        from jax.experimental.shard_map import shard_map
        from concourse.bass2jax import (_bass_exec_p, install_neuronx_cc_hook,
                                        partition_id_tensor)
        install_neuronx_cc_hook()
        self.jax = jax
        self.n_cores = n_cores
        pname = nc.partition_id_tensor.name if nc.partition_id_tensor else None
        in_names, out_names, out_avals, zeros = [], [], [], []
        for alloc in nc.m.functions[0].allocations:
            if not isinstance(alloc, mybir.MemoryLocationSet):
                continue
            name = alloc.memorylocations[0].name
            if alloc.kind == "ExternalInput":
                if name != pname:
                    in_names.append(name)
            elif alloc.kind == "ExternalOutput":
                out_names.append(name)
                shape = tuple(alloc.tensor_shape)
                dt_np = mybir.dt.np(alloc.dtype)
                out_avals.append(jax.core.ShapedArray(shape, dt_np))
                zeros.append(np.zeros(shape, dt_np))
        self.in_names, self.out_names = in_names, out_names
        self.out_avals, self.zeros = out_avals, zeros
        all_names = in_names + out_names + ([pname] if pname else [])

        def _body(*args):
            ops = list(args)
            if pname is not None:
                ops.append(partition_id_tensor())
            return tuple(_bass_exec_p.bind(
                *ops, out_avals=tuple(out_avals), in_names=tuple(all_names),
                out_names=tuple(out_names), lowering_input_output_aliases=(),
                sim_require_finite=True, sim_require_nnan=True, nc=nc))

        mesh = Mesh(np.asarray(jax.devices()[:n_cores]), ("core",))
        spec_in = (PartitionSpec("core"),) * (len(in_names) + len(out_names))
        spec_out = (PartitionSpec("core"),) * len(out_names)
        self.fn = jax.jit(shard_map(_body, mesh=mesh, in_specs=spec_in,
                                    out_specs=spec_out, check_rep=False),
                          keep_unused=True)

    def run(self, in_maps):
        n = self.n_cores
        args = [np.concatenate([np.asarray(in_maps[c][k]) for c in range(n)], axis=0)
                for k in self.in_names]
        args += [np.zeros((n * z.shape[0], *z.shape[1:]), z.dtype)
                 for z in self.zeros]
        outs = self.fn(*args)
        self.jax.block_until_ready(outs)
        return [{k: np.asarray(outs[i]).reshape(n, *self.out_avals[i].shape)[c]
                 for i, k in enumerate(self.out_names)} for c in range(n)]


_runner_cache = {}


def kernel(x, bias, mask, Wq, Wkv, Wo, bo):
    import time as _time
    in_maps = make_in_maps(x, bias, mask, Wq, Wkv, Wo, bo)
    results = None
    # the axon mesh occasionally comes up wedged from a prior aborted
    # session (NRT_EXEC_UNIT_UNRECOVERABLE); retry with a fresh backend
    for attempt in range(3):
        try:
            if "r" not in _runner_cache:
                _runner_cache["r"] = _CachedRunner(_get_nc(1))
            results = _runner_cache["r"].run(in_maps)
            break
        except Exception:
            _runner_cache.pop("r", None)
            try:
                import jax
                jax.clear_caches()
                jax.extend.backend.clear_backends()
            except Exception:
                pass
            _time.sleep(5)
    if results is None:
        res = run_bass_kernel_spmd(_get_nc(1), in_maps, core_ids=list(range(8)))
        results = res.results
    out = np.empty((2, NK, F), dtype=np.float32)
    for c in range(8):
        b, qi = c // 4, c % 4
        out[b, qi * Q:(qi + 1) * Q] = results[c]["out_t"]
    return out


# revision 10
# speedup vs baseline: 1.0304x; 1.0081x over previous
"""Trainium2 Bass kernel for nn_Attention_88441966559243.

Attention with additive bias [B,N,N] and per-key bool mask, fp32.
  B=2, N=2048, QD=1024, HEADS=16, DIM_HEAD=64.

Sharding: 8 cores = (batch b = core//4) x (query slice q0 = (core%4)*512).
Each core computes out[b, q0:q0+512, :]; the host gather is concatenation.
No collectives (an AllGather costs ~100us+ fixed on this fabric).

v7 design:
  - host prep: x arrives pre-transposed (xT [F,NK] bf16) and the
    multiplicative bias arrives as ebiasT = exp(bias^T + maskneg) bf16
    (exp(-30000)==0 makes the key mask exact) -- no on-device transposes.
  - all-bf16 data plane (weights, xT, k^T, v', q^T, exp-weights) with fp32
    PSUM accumulation; bf16 keeps every DMA row >= 512B irrelevant since
    K/V/bias/q never leave SBUF at all.
  - stage C per key-chunk: both sub-heads' sim tiles land in one two-bank
    [128,1024] PSUM tile -> a single ACT exp (bf16) -> {DVE, Pool} in-place
    multiply by ebiasT on disjoint halves -> PE e@v; the ones column of v'
    yields the softmax denominator inside the same accumulation.
  - stage D: out^T SBUF-resident, head pairs packed (contraction 128).

Measured on HW (8 cores, For_i-loop slope timing, NITER=33): ~0.35-0.39 ms
per looped invocation (one-shot likely faster; For_i serializes iterations),
vs 0.747 ms baseline.  Rel err vs fp32 jax reference 5.9e-3.
"""
import sys
for _p in ("/opt/trn_rl_repo", "/root/.axon_site/_ro/trn_rl_repo"):
    if _p not in sys.path:
        sys.path.insert(0, _p)

import os

import numpy as np

import concourse.bass as bass
import concourse.mybir as mybir
from concourse import bacc
from concourse.tile import TileContext
from concourse.masks import make_identity
from concourse.bass_utils import run_bass_kernel_spmd

F = 1024          # feature dim (QD == INNER)
NK = 2048         # keys (full sequence)
Q = 512           # queries per core
H = 16            # heads
D = 64            # head dim
DV = 65           # head dim + ones column
SCALE = D ** -0.5
MASK_NEG = -30000.0

FC = F // 128      # 8 feature chunks
KC = NK // 128     # 16 key chunks
NB = NK // 512     # 4 key 512-blocks

f32 = mybir.dt.float32
fr = mybir.dt.float32r
bf = mybir.dt.bfloat16
AF = mybir.ActivationFunctionType

ABL = os.environ.get("ABL", "")


def build_nc(niter: int = 1):
    nc = bacc.Bacc(None, target_bir_lowering=False)

    xT_in = nc.dram_tensor("xT_in", [F, NK], bf, kind="ExternalInput")
    xqT_in = nc.dram_tensor("xqT_in", [F, Q], bf, kind="ExternalInput")
    ebiasT_in = nc.dram_tensor("ebiasT_in", [NK, Q], bf, kind="ExternalInput")
    wq_in = nc.dram_tensor("wq_in", [F, F], bf, kind="ExternalInput")  # pre-scaled
    wkv_in = nc.dram_tensor("wkv_in", [F, 2 * F], bf, kind="ExternalInput")
    wo_in = nc.dram_tensor("wo_in", [F, F], bf, kind="ExternalInput")
    bo_in = nc.dram_tensor("bo_in", [1, F], fr, kind="ExternalInput")
    out_t = nc.dram_tensor("out_t", [Q, F], f32, kind="ExternalOutput")
    chain_out = (nc.dram_tensor("chain_out", [Q, 256], f32,
                                kind="ExternalOutput") if niter > 1 else None)

    with TileContext(nc) as tc:
        with (
            tc.tile_pool(name="const", bufs=1) as constp,
            tc.tile_pool(name="psu", bufs=2, space="PSUM") as psUp,
        ):
            # ---- constants ----
            ones_f = constp.tile([128, 128], f32)
            nc.vector.memset(ones_f[:, :], 1.0)
            ones_r = constp.tile([128, 128], fr)
            nc.scalar.copy(ones_r[:, :], ones_f[:, :])
            ones_b = constp.tile([128, 128], bf)
            nc.scalar.copy(ones_b[:, :], ones_f[:, :])
            bo_sb = constp.tile([1, F], fr)
            nc.sync.dma_start(bo_sb[:, :], bo_in[:, :])
            bo_rep = constp.tile([128, F], f32)

            def body(_iv=None):
                with (
                    tc.tile_pool(name="xTp", bufs=1) as xTp,
                    tc.tile_pool(name="kTp", bufs=1) as kTp,
                    tc.tile_pool(name="vfp", bufs=1) as vfp,
                    tc.tile_pool(name="qTp", bufs=1) as qTp,
                    tc.tile_pool(name="bTp", bufs=1) as bTp,
                    tc.tile_pool(name="otP", bufs=1) as otPp,
                    tc.tile_pool(name="wop", bufs=1) as wop,
                ):
                    xT = [xTp.tile([128, NK], bf, tag=f"xT{i}", name=f"xT{i}")
                          for i in range(FC)]
                    xqT = [xTp.tile([128, Q], bf, tag=f"xqT{i}", name=f"xqT{i}")
                           for i in range(FC)]
                    kT8 = [kTp.tile([128, NK], bf, tag=f"kT{i}", name=f"kT{i}")
                           for i in range(FC)]
                    vfull = vfp.tile([128, KC * H * DV], bf, name="vfull")
                    qT = [qTp.tile([128, Q], bf, tag=f"qT{i}", name=f"qT{i}")
                          for i in range(FC)]
                    biasT = [bTp.tile([128, Q], bf, tag=f"bT{i}", name=f"bT{i}")
                             for i in range(KC)]
                    otP = [otPp.tile([128, Q], bf, tag=f"ot{i}", name=f"ot{i}")
                           for i in range(H // 2)]
                    wo = [wop.tile([128, F], bf, tag=f"wo{i}", name=f"wo{i}")
                          for i in range(H // 2)]

                    # ---- input loads; A2's inputs (xqT) issue first so
                    # the first matmuls are not gated on the 4MB xT load ----
                    for fc in range(FC):
                        nc.sync.dma_start(xqT[fc][:, :],
                                          xqT_in[fc * 128:(fc + 1) * 128, :])
                    for nb in range(NB):
                        for fc in range(FC):
                            nc.sync.dma_start(
                                xT[fc][:, nb * 512:(nb + 1) * 512],
                                xT_in[fc * 128:(fc + 1) * 128,
                                      nb * 512:(nb + 1) * 512])
                    for kc in range(KC):
                        nc.sync.dma_start(biasT[kc][:, :],
                                          ebiasT_in[kc * 128:(kc + 1) * 128, :])

                    # ======== stage A ========
                    with (
                        tc.tile_pool(name="wkp", bufs=8) as wkp,
                        tc.tile_pool(name="wqv", bufs=8) as wqvp,
                        tc.tile_pool(name="psa", bufs=6, space="PSUM") as psA,
                    ):
                        wq = [wqvp.tile([128, F], bf, tag="w", name="w")
                              for _ in range(FC)]
                        for fc in range(FC):
                            nc.sync.dma_start(wq[fc][:, :],
                                              wq_in[fc * 128:(fc + 1) * 128, :])
                        wk = [wkp.tile([128, F], bf, tag="wk", name="wk")
                              for _ in range(FC)]
                        for fc in range(FC):
                            nc.sync.dma_start(
                                wk[fc][:, :], wkv_in[fc * 128:(fc + 1) * 128, 0:F])
                        for i in range(H // 2):
                            nc.sync.dma_start(wo[i][:, :],
                                              wo_in[i * 128:(i + 1) * 128, :])

                        # A2: qT = Wq^T @ xqT (Wq pre-scaled on host)
                        for m in range(FC):
                            ps = psA.tile([128, 512], f32, name="psa")
                            for fc in range(FC):
                                nc.tensor.matmul(
                                    ps[:, :],
                                    wq[fc][:, m * 128:(m + 1) * 128],
                                    xqT[fc][:, :],
                                    start=(fc == 0), stop=(fc == FC - 1))
                            nc.scalar.copy(qT[m][:, :], ps[:, :])
                        # bo broadcast (PE free)
                        for nb2 in range(2):
                            ps = psA.tile([128, 512], f32, name="psa")
                            nc.tensor.matmul(ps[:, :], ones_r[0:1, 0:128],
                                             bo_sb[0:1, nb2 * 512:(nb2 + 1) * 512],
                                             start=True, stop=True)
                            nc.scalar.copy(bo_rep[:, nb2 * 512:(nb2 + 1) * 512],
                                           ps[:, :])

                        # A3: kT8[m] = (Wk^T @ xT) rows of head-pair m;
                        # key-block outer: block 0 starts once 0.5MB of xT
                        # has landed instead of the full 4MB
                        for nb in range(NB):
                            for m in range(FC):
                                ps = psA.tile([128, 512], f32, name="psa")
                                for fc in range(FC):
                                    nc.tensor.matmul(
                                        ps[:, :],
                                        wk[fc][:, m * 128:(m + 1) * 128],
                                        xT[fc][:, nb * 512:(nb + 1) * 512],
                                        start=(fc == 0), stop=(fc == FC - 1))
                                nc.scalar.copy(kT8[m][:, nb * 512:(nb + 1) * 512],
                                               ps[:, :])

                        # A4: vfull = [x @ Wv | 1] (keys-major, bf16)
                        wv = [wqvp.tile([128, F], bf, tag="w", name="w")
                              for _ in range(FC)]
                        for fc in range(FC):
                            nc.sync.dma_start(
                                wv[fc][:, :],
                                wkv_in[fc * 128:(fc + 1) * 128, F:2 * F])
                        for kc in range(KC):
                            vrow = vfull[:, kc * H * DV:(kc + 1) * H * DV]
                            for half in range(2):
                                ps = psA.tile([128, 512], f32, name="psa")
                                for fc in range(FC):
                                    nc.tensor.matmul(
                                        ps[:, :],
                                        xT[fc][:, kc * 128:(kc + 1) * 128],
                                        wv[fc][:, half * 512:(half + 1) * 512],
                                        start=(fc == 0), stop=(fc == FC - 1))
                                dst = vrow[:, half * 8 * DV:(half + 1) * 8 * DV] \
                                    .rearrange("p (h x) -> p h x", x=DV)[:, :, 0:64]
                                nc.scalar.copy(
                                    dst,
                                    ps[:, :].rearrange("p (h d) -> p h d", d=64))
                            ones_dst = vrow.rearrange(
                                "p (h x) -> p h x", x=DV)[:, :, 64:65]
                            nc.vector.tensor_copy(
                                ones_dst,
                                ones_b[:, 0:H].rearrange("p (a b) -> p a b", b=1))

                    # ======== stage C: attention, head pairs ========
                    with (
                        tc.tile_pool(name="et", bufs=6) as ep,
                        tc.tile_pool(name="dsb", bufs=2) as dsbp,
                        tc.tile_pool(name="rrep", bufs=2) as rrepp,
                        tc.tile_pool(name="psc", bufs=3, space="PSUM") as psC,
                    ):
                        for hp in range(H // 2):
                            psU2 = [psUp.tile([DV, 512], f32, name="psu")
                                    for _ in range(2)]
                            # lag e@v behind the sim->exp->mul chain so the
                            # in-order PE queue never waits on it.
                            pending = []

                            def drain_av(upto):
                                while pending and pending[0][0] <= upto:
                                    kc0, eT2_ = pending.pop(0)
                                    for sub in range(2):
                                        nc.tensor.matmul(
                                            psU2[sub][:, :],
                                            vfull[:, kc0 * H * DV +
                                                  (2 * hp + sub) * DV:
                                                  kc0 * H * DV +
                                                  (2 * hp + sub + 1) * DV],
                                            eT2_[:, sub * 512:(sub + 1) * 512],
                                            start=(kc0 == 0),
                                            stop=(kc0 == KC - 1))

                            for kc in range(KC):
                                # both sub-heads' sim tiles land in one
                                # two-bank PSUM tile -> a single ACT exp
                                # instruction covers 1024 columns
                                ps2 = psC.tile([128, 1024], f32, name="psc")
                                for sub in range(2):
                                    po = sub * 64
                                    nc.tensor.matmul(
                                        ps2[:, sub * 512:(sub + 1) * 512],
                                        kT8[hp][po:po + 64,
                                                kc * 128:(kc + 1) * 128],
                                        qT[hp][po:po + 64, :],
                                        start=True, stop=True)
                                eT2 = ep.tile([128, 1024], bf, name="eT")
                                nc.scalar.activation(
                                    eT2[:, :], ps2[:, :], AF.Exp, scale=1.0)
                                # ebias multiply split across the two free
                                # vector engines (disjoint halves)
                                nc.vector.tensor_mul(
                                    eT2[:, 0:512], eT2[:, 0:512],
                                    biasT[kc][:, :])
                                eng2 = nc.vector if ABL == "nogp" else nc.gpsimd
                                eng2.tensor_mul(
                                    eT2[:, 512:1024], eT2[:, 512:1024],
                                    biasT[kc][:, :])
                                pending.append((kc, eT2))
                                if kc >= 4:
                                    drain_av(kc - 3)
                            drain_av(KC)
                            for sub in range(2):
                                psU = psU2[sub]
                                Dsb = dsbp.tile([DV, 512], fr, name="Dsb")
                                nc.scalar.copy(Dsb[64:65, :], psU[64:65, :])
                                psR = psC.tile([128, 1024], f32, name="psc")
                                nc.tensor.matmul(psR[0:64, 0:512],
                                                 ones_r[64:65, 0:64],
                                                 Dsb[64:65, :],
                                                 start=True, stop=True)
                                rrep = rrepp.tile([64, 512], f32, name="rrep")
                                nc.vector.reciprocal_approx_fast(
                                    out=rrep[:, :], in_=psR[0:64, 0:512])
                                nc.vector.tensor_mul(
                                    otP[hp][sub * 64:(sub + 1) * 64, :],
                                    psU[0:64, :], rrep[:, :])

                    # ======== stage D (SBUF-resident, head-pair packed) ======
                    with (
                        tc.tile_pool(name="fin", bufs=3) as finp,
                        tc.tile_pool(name="psd", bufs=3, space="PSUM") as psD,
                    ):
                        for mc in range(4):
                            for nb2 in range(2):
                                psF = psD.tile([128, 512], f32, name="psd")
                                for i in range(H // 2):
                                    nc.tensor.matmul(
                                        psF[:, :],
                                        otP[i][:, mc * 128:(mc + 1) * 128],
                                        wo[i][:, nb2 * 512:(nb2 + 1) * 512],
                                        start=(i == 0), stop=(i == H // 2 - 1))
                                fin = finp.tile([128, 512], f32, name="fin")
                                nc.vector.tensor_add(
                                    fin[:, :], psF[:, :],
                                    bo_rep[:, nb2 * 512:(nb2 + 1) * 512])
                                nc.sync.dma_start(
                                    out_t[mc * 128:(mc + 1) * 128,
                                          nb2 * 512:(nb2 + 1) * 512],
                                    fin[:, :])

            def chain_guard():
                # timing builds only: read back a slice that touches every
                # out_t store tile and expose it as a second output, so the
                # compiler cannot dead-code-eliminate identical unrolled
                # bodies (pure-SBUF bodies otherwise collapse to one).
                with tc.tile_pool(name="chain", bufs=2) as chp:
                    for mc in range(4):
                        sN = chp.tile([128, 256], f32, name="chs")
                        nc.sync.dma_start(sN[:, :],
                                          out_t[mc * 128:(mc + 1) * 128,
                                                384:640])
                        nc.sync.dma_start(
                            chain_out[mc * 128:(mc + 1) * 128, :], sN[:, :])

            mode = os.environ.get("TMODE", "fori")
            if niter == 1:
                body()
            elif mode == "unroll":
                for _ in range(niter):
                    body()
                    chain_guard()
            else:
                with tc.For_i(0, niter, 1) as iv:
                    body(iv)

    nc.finalize()
    return nc


_nc_cache = {}


def _get_nc(niter=1):
    if niter not in _nc_cache:
        _nc_cache[niter] = build_nc(niter)
    return _nc_cache[niter]


def make_in_maps(x, bias, mask, Wq, Wkv, Wo, bo):
    bf_np = mybir.dt.np(bf)
    x = np.asarray(x, dtype=np.float32)
    bias = np.asarray(bias, dtype=np.float32)
    mask = np.asarray(mask)
    wq_scaled = np.ascontiguousarray(
        (np.asarray(Wq, dtype=np.float32) * np.float32(SCALE)).astype(bf_np))
    wkv_b = np.ascontiguousarray(np.asarray(Wkv, dtype=np.float32).astype(bf_np))
    wo_b = np.ascontiguousarray(np.asarray(Wo, dtype=np.float32).astype(bf_np))
    bo_f = np.ascontiguousarray(np.asarray(bo, dtype=np.float32).reshape(1, F))
    xT_b, ebias_b = {}, {}
    for b in range(2):
        xT_b[b] = np.ascontiguousarray(x[b].T.astype(bf_np))
        maskneg = np.where(mask[b], 0.0, MASK_NEG).astype(np.float32)
        # ebiasT[key, query] = exp(bias + mask) per batch, transposed
        ebias_b[b] = np.exp(bias[b].T + maskneg[:, None]).astype(bf_np)
    in_maps = []
    for c in range(8):
        b, qi = c // 4, c % 4
        q0 = qi * Q
        in_maps.append({
            "xT_in": xT_b[b],
            "xqT_in": np.ascontiguousarray(xT_b[b][:, q0:q0 + Q]),
            "ebiasT_in": np.ascontiguousarray(ebias_b[b][:, q0:q0 + Q]),
            "wq_in": wq_scaled,
            "wkv_in": wkv_b,
            "wo_in": wo_b,
            "bo_in": bo_f,
        })
    return in_maps


class _CachedRunner:
    """Jit the NEFF-backed executable once; repeat kernel() calls then skip
    the ~40s relower/recompile and run in ~0.1s."""

    def __init__(self, nc, n_cores=8):
        import jax
        from jax.sharding import Mesh, PartitionSpec
        from jax.experimental.shard_map import shard_map
        from concourse.bass2jax import (_bass_exec_p, install_neuronx_cc_hook,
                                        partition_id_tensor)
        install_neuronx_cc_hook()
        self.jax = jax
        self.n_cores = n_cores
        pname = nc.partition_id_tensor.name if nc.partition_id_tensor else None
        in_names, out_names, out_avals, zeros = [], [], [], []
        for alloc in nc.m.functions[0].allocations:
            if not isinstance(alloc, mybir.MemoryLocationSet):
                continue
            name = alloc.memorylocations[0].name
            if alloc.kind == "ExternalInput":
                if name != pname:
                    in_names.append(name)
            elif alloc.kind == "ExternalOutput":
                out_names.append(name)
                shape = tuple(alloc.tensor_shape)
                dt_np = mybir.dt.np(alloc.dtype)
                out_avals.append(jax.core.ShapedArray(shape, dt_np))
                zeros.append(np.zeros(shape, dt_np))
        self.in_names, self.out_names = in_names, out_names
        self.out_avals, self.zeros = out_avals, zeros
        all_names = in_names + out_names + ([pname] if pname else [])

        def _body(*args):
            ops = list(args)
            if pname is not None:
                ops.append(partition_id_tensor())
            return tuple(_bass_exec_p.bind(
                *ops, out_avals=tuple(out_avals), in_names=tuple(all_names),
                out_names=tuple(out_names), lowering_input_output_aliases=(),
                sim_require_finite=True, sim_require_nnan=True, nc=nc))

        mesh = Mesh(np.asarray(jax.devices()[:n_cores]), ("core",))
        spec_in = (PartitionSpec("core"),) * (len(in_names) + len(out_names))
        spec_out = (PartitionSpec("core"),) * len(out_names)
        self.fn = jax.jit(shard_map(_body, mesh=mesh, in_specs=spec_in,
                                    out_specs=spec_out, check_rep=False),
                          keep_unused=True)

    def run(self, in_maps):
        n = self.n_cores
        args = [np.concatenate([np.asarray(in_maps[c][k]) for c in range(n)], axis=0)
                for k in self.in_names]
        args += [np.zeros((n * z.shape[0], *z.shape[1:]), z.dtype)
                 for z in self.zeros]
        outs = self.fn(*args)
        self.jax.block_until_ready(outs)
        return [{k: np.asarray(outs[i]).reshape(n, *self.out_avals[i].shape)[c]
                 for i, k in enumerate(self.out_names)} for c in range(n)]


_runner_cache = {}


def kernel(x, bias, mask, Wq, Wkv, Wo, bo):
    import time as _time
    in_maps = make_in_maps(x, bias, mask, Wq, Wkv, Wo, bo)
    results = None
    # the axon mesh occasionally comes up wedged from a prior aborted
    # session (NRT_EXEC_UNIT_UNRECOVERABLE); retry with a fresh backend
    for attempt in range(3):
        try:
            if "r" not in _runner_cache:
                _runner_cache["r"] = _CachedRunner(_get_nc(1))
            results = _runner_cache["r"].run(in_maps)
            break
        except Exception:
            _runner_cache.pop("r", None)
            try:
                import jax
                jax.clear_caches()
                jax.extend.backend.clear_backends()
            except Exception:
                pass
            _time.sleep(5)
    if results is None:
        res = run_bass_kernel_spmd(_get_nc(1), in_maps, core_ids=list(range(8)))
        results = res.results
    out = np.empty((2, NK, F), dtype=np.float32)
    for c in range(8):
        b, qi = c // 4, c % 4
        out[b, qi * Q:(qi + 1) * Q] = results[c]["out_t"]
    return out


# revision 11
# speedup vs baseline: 1.1198x; 1.0868x over previous
"""Trainium2 Bass kernel for nn_Attention_88441966559243.

Attention with additive bias [B,N,N] and per-key bool mask, fp32.
  B=2, N=2048, QD=1024, HEADS=16, DIM_HEAD=64.

Sharding: 8 cores = (batch b = core//4) x (query slice q0 = (core%4)*512).
Each core computes out[b, q0:q0+512, :]; the host gather is concatenation.
No collectives (an AllGather costs ~100us+ fixed on this fabric).

v7 design:
  - host prep: x arrives pre-transposed (xT [F,NK] bf16) and the
    multiplicative bias arrives as ebiasT = exp(bias^T + maskneg) bf16
    (exp(-30000)==0 makes the key mask exact) -- no on-device transposes.
  - all-bf16 data plane (weights, xT, k^T, v', q^T, exp-weights) with fp32
    PSUM accumulation; bf16 keeps every DMA row >= 512B irrelevant since
    K/V/bias/q never leave SBUF at all.
  - stage C per key-chunk: both sub-heads' sim tiles land in one two-bank
    [128,1024] PSUM tile -> a single ACT exp (bf16) -> {DVE, Pool} in-place
    multiply by ebiasT on disjoint halves -> PE e@v; the ones column of v'
    yields the softmax denominator inside the same accumulation.
  - stage D: out^T SBUF-resident, head pairs packed (contraction 128).

Measured on HW (8 cores, For_i-loop slope timing, NITER=33, several runs):
~0.34-0.39 ms per looped invocation -- the loop re-pays the 13MB input load
every iteration, so a one-shot invocation is faster -- vs 0.747 ms baseline.
Rel err vs fp32 jax reference 5.9e-3.
"""
import sys
for _p in ("/opt/trn_rl_repo", "/root/.axon_site/_ro/trn_rl_repo"):
    if _p not in sys.path:
        sys.path.insert(0, _p)

import os

import numpy as np

import concourse.bass as bass
import concourse.mybir as mybir
from concourse import bacc
from concourse.tile import TileContext
from concourse.masks import make_identity
from concourse.bass_utils import run_bass_kernel_spmd

F = 1024          # feature dim (QD == INNER)
NK = 2048         # keys (full sequence)
Q = 512           # queries per core
H = 16            # heads
D = 64            # head dim
DV = 65           # head dim + ones column
SCALE = D ** -0.5
MASK_NEG = -30000.0

FC = F // 128      # 8 feature chunks
KC = NK // 128     # 16 key chunks
NB = NK // 512     # 4 key 512-blocks

f32 = mybir.dt.float32
fr = mybir.dt.float32r
bf = mybir.dt.bfloat16
AF = mybir.ActivationFunctionType

ABL = os.environ.get("ABL", "")


def build_nc(niter: int = 1):
    nc = bacc.Bacc(None, target_bir_lowering=False)

    xT_in = nc.dram_tensor("xT_in", [F, NK], bf, kind="ExternalInput")
    xqT_in = nc.dram_tensor("xqT_in", [F, Q], bf, kind="ExternalInput")
    ebiasT_in = nc.dram_tensor("ebiasT_in", [NK, Q], bf, kind="ExternalInput")
    wq_in = nc.dram_tensor("wq_in", [F, F], bf, kind="ExternalInput")  # pre-scaled
    wkv_in = nc.dram_tensor("wkv_in", [F, 2 * F], bf, kind="ExternalInput")
    wo_in = nc.dram_tensor("wo_in", [F, F], bf, kind="ExternalInput")
    bo_in = nc.dram_tensor("bo_in", [1, F], fr, kind="ExternalInput")
    out_t = nc.dram_tensor("out_t", [Q, F], f32, kind="ExternalOutput")
    chain_out = (nc.dram_tensor("chain_out", [Q, 256], f32,
                                kind="ExternalOutput") if niter > 1 else None)

    with TileContext(nc) as tc:
        with (
            tc.tile_pool(name="const", bufs=1) as constp,
            tc.tile_pool(name="psu", bufs=2, space="PSUM") as psUp,
        ):
            # ---- constants ----
            ones_f = constp.tile([128, 128], f32)
            nc.vector.memset(ones_f[:, :], 1.0)
            ones_r = constp.tile([128, 128], fr)
            nc.scalar.copy(ones_r[:, :], ones_f[:, :])
            ones_b = constp.tile([128, 128], bf)
            nc.scalar.copy(ones_b[:, :], ones_f[:, :])
            bo_sb = constp.tile([1, F], fr)
            nc.sync.dma_start(bo_sb[:, :], bo_in[:, :])
            bo_rep = constp.tile([128, F], f32)

            def body(_iv=None):
                with (
                    tc.tile_pool(name="xTp", bufs=1) as xTp,
                    tc.tile_pool(name="kTp", bufs=1) as kTp,
                    tc.tile_pool(name="vfp", bufs=1) as vfp,
                    tc.tile_pool(name="qTp", bufs=1) as qTp,
                    tc.tile_pool(name="bTp", bufs=1) as bTp,
                    tc.tile_pool(name="otP", bufs=1) as otPp,
                    tc.tile_pool(name="wop", bufs=1) as wop,
                ):
                    xT = [xTp.tile([128, NK], bf, tag=f"xT{i}", name=f"xT{i}")
                          for i in range(FC)]
                    xqT = [xTp.tile([128, Q], bf, tag=f"xqT{i}", name=f"xqT{i}")
                           for i in range(FC)]
                    kT8 = [kTp.tile([128, NK], bf, tag=f"kT{i}", name=f"kT{i}")
                           for i in range(FC)]
                    vfull = vfp.tile([128, KC * H * DV], bf, name="vfull")
                    qT = [qTp.tile([128, Q], bf, tag=f"qT{i}", name=f"qT{i}")
                          for i in range(FC)]
                    biasT = [bTp.tile([128, Q], bf, tag=f"bT{i}", name=f"bT{i}")
                             for i in range(KC)]
                    otP = [otPp.tile([128, Q], bf, tag=f"ot{i}", name=f"ot{i}")
                           for i in range(H // 2)]
                    wo = [wop.tile([128, F], bf, tag=f"wo{i}", name=f"wo{i}")
                          for i in range(H // 2)]

                    # ---- input loads; A2's inputs (xqT) issue first so
                    # the first matmuls are not gated on the 4MB xT load ----
                    for fc in range(FC):
                        nc.sync.dma_start(xqT[fc][:, :],
                                          xqT_in[fc * 128:(fc + 1) * 128, :])
                    for nb in range(NB):
                        for fc in range(FC):
                            nc.sync.dma_start(
                                xT[fc][:, nb * 512:(nb + 1) * 512],
                                xT_in[fc * 128:(fc + 1) * 128,
                                      nb * 512:(nb + 1) * 512])
                    for kc in range(KC):
                        nc.sync.dma_start(biasT[kc][:, :],
                                          ebiasT_in[kc * 128:(kc + 1) * 128, :])

                    # ======== stage A ========
                    with (
                        tc.tile_pool(name="wkp", bufs=8) as wkp,
                        tc.tile_pool(name="wqv", bufs=8) as wqvp,
                        tc.tile_pool(name="psa", bufs=6, space="PSUM") as psA,
                    ):
                        wq = [wqvp.tile([128, F], bf, tag="w", name="w")
                              for _ in range(FC)]
                        for fc in range(FC):
                            nc.sync.dma_start(wq[fc][:, :],
                                              wq_in[fc * 128:(fc + 1) * 128, :])
                        wk = [wkp.tile([128, F], bf, tag="wk", name="wk")
                              for _ in range(FC)]
                        for fc in range(FC):
                            nc.sync.dma_start(
                                wk[fc][:, :], wkv_in[fc * 128:(fc + 1) * 128, 0:F])
                        for i in range(H // 2):
                            nc.sync.dma_start(wo[i][:, :],
                                              wo_in[i * 128:(i + 1) * 128, :])

                        # A2: qT = Wq^T @ xqT (Wq pre-scaled on host)
                        for m in range(FC):
                            ps = psA.tile([128, 512], f32, name="psa")
                            for fc in range(FC):
                                nc.tensor.matmul(
                                    ps[:, :],
                                    wq[fc][:, m * 128:(m + 1) * 128],
                                    xqT[fc][:, :],
                                    start=(fc == 0), stop=(fc == FC - 1))
                            nc.scalar.copy(qT[m][:, :], ps[:, :])
                        # bo broadcast (PE free)
                        for nb2 in range(2):
                            ps = psA.tile([128, 512], f32, name="psa")
                            nc.tensor.matmul(ps[:, :], ones_r[0:1, 0:128],
                                             bo_sb[0:1, nb2 * 512:(nb2 + 1) * 512],
                                             start=True, stop=True)
                            nc.scalar.copy(bo_rep[:, nb2 * 512:(nb2 + 1) * 512],
                                           ps[:, :])

                        # A3: kT8[m] = (Wk^T @ xT) rows of head-pair m;
                        # key-block outer: block 0 starts once 0.5MB of xT
                        # has landed instead of the full 4MB
                        for nb in range(NB):
                            for m in range(FC):
                                ps = psA.tile([128, 512], f32, name="psa")
                                for fc in range(FC):
                                    nc.tensor.matmul(
                                        ps[:, :],
                                        wk[fc][:, m * 128:(m + 1) * 128],
                                        xT[fc][:, nb * 512:(nb + 1) * 512],
                                        start=(fc == 0), stop=(fc == FC - 1))
                                nc.scalar.copy(kT8[m][:, nb * 512:(nb + 1) * 512],
                                               ps[:, :])

                        # A4: vfull = [x @ Wv | 1] (keys-major, bf16)
                        wv = [wqvp.tile([128, F], bf, tag="w", name="w")
                              for _ in range(FC)]
                        for fc in range(FC):
                            nc.sync.dma_start(
                                wv[fc][:, :],
                                wkv_in[fc * 128:(fc + 1) * 128, F:2 * F])
                        for kc in range(KC):
                            vrow = vfull[:, kc * H * DV:(kc + 1) * H * DV]
                            for half in range(2):
                                ps = psA.tile([128, 512], f32, name="psa")
                                for fc in range(FC):
                                    nc.tensor.matmul(
                                        ps[:, :],
                                        xT[fc][:, kc * 128:(kc + 1) * 128],
                                        wv[fc][:, half * 512:(half + 1) * 512],
                                        start=(fc == 0), stop=(fc == FC - 1))
                                dst = vrow[:, half * 8 * DV:(half + 1) * 8 * DV] \
                                    .rearrange("p (h x) -> p h x", x=DV)[:, :, 0:64]
                                nc.scalar.copy(
                                    dst,
                                    ps[:, :].rearrange("p (h d) -> p h d", d=64))
                            ones_dst = vrow.rearrange(
                                "p (h x) -> p h x", x=DV)[:, :, 64:65]
                            nc.vector.tensor_copy(
                                ones_dst,
                                ones_b[:, 0:H].rearrange("p (a b) -> p a b", b=1))

                    # ======== stage C: attention, head pairs ========
                    with (
                        tc.tile_pool(name="et", bufs=6) as ep,
                        tc.tile_pool(name="dsb", bufs=2) as dsbp,
                        tc.tile_pool(name="rrep", bufs=2) as rrepp,
                        tc.tile_pool(name="psc", bufs=3, space="PSUM") as psC,
                    ):
                        for hp in range(H // 2):
                            psU2 = [psUp.tile([DV, 512], f32, name="psu")
                                    for _ in range(2)]
                            # lag e@v behind the sim->exp->mul chain so the
                            # in-order PE queue never waits on it.
                            pending = []

                            def drain_av(upto):
                                while pending and pending[0][0] <= upto:
                                    kc0, eT2_ = pending.pop(0)
                                    for sub in range(2):
                                        nc.tensor.matmul(
                                            psU2[sub][:, :],
                                            vfull[:, kc0 * H * DV +
                                                  (2 * hp + sub) * DV:
                                                  kc0 * H * DV +
                                                  (2 * hp + sub + 1) * DV],
                                            eT2_[:, sub * 512:(sub + 1) * 512],
                                            start=(kc0 == 0),
                                            stop=(kc0 == KC - 1))

                            for kc in range(KC):
                                # both sub-heads' sim tiles land in one
                                # two-bank PSUM tile -> a single ACT exp
                                # instruction covers 1024 columns
                                ps2 = psC.tile([128, 1024], f32, name="psc")
                                for sub in range(2):
                                    po = sub * 64
                                    nc.tensor.matmul(
                                        ps2[:, sub * 512:(sub + 1) * 512],
                                        kT8[hp][po:po + 64,
                                                kc * 128:(kc + 1) * 128],
                                        qT[hp][po:po + 64, :],
                                        start=True, stop=True)
                                eT2 = ep.tile([128, 1024], bf, name="eT")
                                nc.scalar.activation(
                                    eT2[:, :], ps2[:, :], AF.Exp, scale=1.0)
                                # ebias multiply split across the two free
                                # vector engines (disjoint halves)
                                nc.vector.tensor_mul(
                                    eT2[:, 0:512], eT2[:, 0:512],
                                    biasT[kc][:, :])
                                eng2 = nc.vector if ABL == "nogp" else nc.gpsimd
                                eng2.tensor_mul(
                                    eT2[:, 512:1024], eT2[:, 512:1024],
                                    biasT[kc][:, :])
                                pending.append((kc, eT2))
                                if kc >= 4:
                                    drain_av(kc - 3)
                            drain_av(KC)
                            for sub in range(2):
                                psU = psU2[sub]
                                Dsb = dsbp.tile([DV, 512], fr, name="Dsb")
                                nc.scalar.copy(Dsb[64:65, :], psU[64:65, :])
                                psR = psC.tile([128, 1024], f32, name="psc")
                                nc.tensor.matmul(psR[0:64, 0:512],
                                                 ones_r[64:65, 0:64],
                                                 Dsb[64:65, :],
                                                 start=True, stop=True)
                                rrep = rrepp.tile([64, 512], f32, name="rrep")
                                nc.vector.reciprocal_approx_fast(
                                    out=rrep[:, :], in_=psR[0:64, 0:512])
                                nc.vector.tensor_mul(
                                    otP[hp][sub * 64:(sub + 1) * 64, :],
                                    psU[0:64, :], rrep[:, :])

                    # ======== stage D (SBUF-resident, head-pair packed) ======
                    with (
                        tc.tile_pool(name="fin", bufs=3) as finp,
                        tc.tile_pool(name="psd", bufs=3, space="PSUM") as psD,
                    ):
                        for mc in range(4):
                            for nb2 in range(2):
                                psF = psD.tile([128, 512], f32, name="psd")
                                for i in range(H // 2):
                                    nc.tensor.matmul(
                                        psF[:, :],
                                        otP[i][:, mc * 128:(mc + 1) * 128],
                                        wo[i][:, nb2 * 512:(nb2 + 1) * 512],
                                        start=(i == 0), stop=(i == H // 2 - 1))
                                fin = finp.tile([128, 512], f32, name="fin")
                                nc.vector.tensor_add(
                                    fin[:, :], psF[:, :],
                                    bo_rep[:, nb2 * 512:(nb2 + 1) * 512])
                                nc.sync.dma_start(
                                    out_t[mc * 128:(mc + 1) * 128,
                                          nb2 * 512:(nb2 + 1) * 512],
                                    fin[:, :])

            def chain_guard():
                # timing builds only: read back a slice that touches every
                # out_t store tile and expose it as a second output, so the
                # compiler cannot dead-code-eliminate identical unrolled
                # bodies (pure-SBUF bodies otherwise collapse to one).
                with tc.tile_pool(name="chain", bufs=2) as chp:
                    for mc in range(4):
                        sN = chp.tile([128, 256], f32, name="chs")
                        nc.sync.dma_start(sN[:, :],
                                          out_t[mc * 128:(mc + 1) * 128,
                                                384:640])
                        nc.sync.dma_start(
                            chain_out[mc * 128:(mc + 1) * 128, :], sN[:, :])

            mode = os.environ.get("TMODE", "fori")
            if niter == 1:
                body()
            elif mode == "unroll":
                for _ in range(niter):
                    body()
                    chain_guard()
            else:
                with tc.For_i(0, niter, 1) as iv:
                    body(iv)

    nc.finalize()
    return nc


_nc_cache = {}


def _get_nc(niter=1):
    if niter not in _nc_cache:
        _nc_cache[niter] = build_nc(niter)
    return _nc_cache[niter]


def make_in_maps(x, bias, mask, Wq, Wkv, Wo, bo):
    bf_np = mybir.dt.np(bf)
    x = np.asarray(x, dtype=np.float32)
    bias = np.asarray(bias, dtype=np.float32)
    mask = np.asarray(mask)
    wq_scaled = np.ascontiguousarray(
        (np.asarray(Wq, dtype=np.float32) * np.float32(SCALE)).astype(bf_np))
    wkv_b = np.ascontiguousarray(np.asarray(Wkv, dtype=np.float32).astype(bf_np))
    wo_b = np.ascontiguousarray(np.asarray(Wo, dtype=np.float32).astype(bf_np))
    bo_f = np.ascontiguousarray(np.asarray(bo, dtype=np.float32).reshape(1, F))
    xT_b, ebias_b = {}, {}
    for b in range(2):
        xT_b[b] = np.ascontiguousarray(x[b].T.astype(bf_np))
        maskneg = np.where(mask[b], 0.0, MASK_NEG).astype(np.float32)
        # ebiasT[key, query] = exp(bias + mask) per batch, transposed
        ebias_b[b] = np.exp(bias[b].T + maskneg[:, None]).astype(bf_np)
    in_maps = []
    for c in range(8):
        b, qi = c // 4, c % 4
        q0 = qi * Q
        in_maps.append({
            "xT_in": xT_b[b],
            "xqT_in": np.ascontiguousarray(xT_b[b][:, q0:q0 + Q]),
            "ebiasT_in": np.ascontiguousarray(ebias_b[b][:, q0:q0 + Q]),
            "wq_in": wq_scaled,
            "wkv_in": wkv_b,
            "wo_in": wo_b,
            "bo_in": bo_f,
        })
    return in_maps


class _CachedRunner:
    """Jit the NEFF-backed executable once; repeat kernel() calls then skip
    the ~40s relower/recompile and run in ~0.1s."""

    def __init__(self, nc, n_cores=8):
        import jax
        from jax.sharding import Mesh, PartitionSpec
        from jax.experimental.shard_map import shard_map
        from concourse.bass2jax import (_bass_exec_p, install_neuronx_cc_hook,
                                        partition_id_tensor)
        install_neuronx_cc_hook()
        self.jax = jax
        self.n_cores = n_cores
        pname = nc.partition_id_tensor.name if nc.partition_id_tensor else None
        in_names, out_names, out_avals, zeros = [], [], [], []
        for alloc in nc.m.functions[0].allocations:
            if not isinstance(alloc, mybir.MemoryLocationSet):
                continue
            name = alloc.memorylocations[0].name
            if alloc.kind == "ExternalInput":
                if name != pname:
                    in_names.append(name)
            elif alloc.kind == "ExternalOutput":
                out_names.append(name)
                shape = tuple(alloc.tensor_shape)
                dt_np = mybir.dt.np(alloc.dtype)
                out_avals.append(jax.core.ShapedArray(shape, dt_np))
                zeros.append(np.zeros(shape, dt_np))
        self.in_names, self.out_names = in_names, out_names
        self.out_avals, self.zeros = out_avals, zeros
        all_names = in_names + out_names + ([pname] if pname else [])

        def _body(*args):
            ops = list(args)
            if pname is not None:
                ops.append(partition_id_tensor())
            return tuple(_bass_exec_p.bind(
                *ops, out_avals=tuple(out_avals), in_names=tuple(all_names),
                out_names=tuple(out_names), lowering_input_output_aliases=(),
                sim_require_finite=True, sim_require_nnan=True, nc=nc))

        mesh = Mesh(np.asarray(jax.devices()[:n_cores]), ("core",))
        spec_in = (PartitionSpec("core"),) * (len(in_names) + len(out_names))
        spec_out = (PartitionSpec("core"),) * len(out_names)
        self.fn = jax.jit(shard_map(_body, mesh=mesh, in_specs=spec_in,
                                    out_specs=spec_out, check_rep=False),
                          keep_unused=True)

    def run(self, in_maps):
        n = self.n_cores
        args = [np.concatenate([np.asarray(in_maps[c][k]) for c in range(n)], axis=0)
                for k in self.in_names]
        args += [np.zeros((n * z.shape[0], *z.shape[1:]), z.dtype)
                 for z in self.zeros]
        outs = self.fn(*args)
        self.jax.block_until_ready(outs)
        return [{k: np.asarray(outs[i]).reshape(n, *self.out_avals[i].shape)[c]
                 for i, k in enumerate(self.out_names)} for c in range(n)]


_runner_cache = {}


def kernel(x, bias, mask, Wq, Wkv, Wo, bo):
    import time as _time
    in_maps = make_in_maps(x, bias, mask, Wq, Wkv, Wo, bo)
    results = None
    # the axon mesh occasionally comes up wedged from a prior aborted
    # session (NRT_EXEC_UNIT_UNRECOVERABLE); retry with a fresh backend
    for attempt in range(3):
        try:
            if "r" not in _runner_cache:
                _runner_cache["r"] = _CachedRunner(_get_nc(1))
            results = _runner_cache["r"].run(in_maps)
            break
        except Exception:
            _runner_cache.pop("r", None)
            try:
                import jax
                jax.clear_caches()
                jax.extend.backend.clear_backends()
            except Exception:
                pass
            _time.sleep(5)
    if results is None:
        res = run_bass_kernel_spmd(_get_nc(1), in_maps, core_ids=list(range(8)))
        results = res.results
    out = np.empty((2, NK, F), dtype=np.float32)
    for c in range(8):
        b, qi = c // 4, c % 4
        out[b, qi * Q:(qi + 1) * Q] = results[c]["out_t"]
    return out
